# revision 27
# baseline (speedup 1.0000x reference)
"""Multi-head SwiGLU feed-forward (nn_MultiHeadFeedForward) Trainium2 kernel.

Math (per head h of 16, head_dim d=128, ffn f=512):
    g = x_h @ gate_w[h]      # [T,128]@[128,512]
    u = x_h @ up_w[h]
    out_h = (silu(g)*u) @ down_w[h]   # [T,512]@[512,128]

Sharding: 2 heads per core across 8 cores (no cross-core communication).
On-chip layout is feature-major ("transposed"): activations live as
[feature, token] tiles so every matmul contracts along the partition dim
without any on-chip transposes.  The host pre-transposes x into
xT[h, d, t] per core and un-transposes the output.
"""

import os
import sys

import numpy as np

for _p in ("/opt/trn_rl_repo",):
    if _p not in sys.path and os.path.isdir(_p):
        sys.path.insert(0, _p)

import concourse.bass as bass
import concourse.mybir as mybir
from concourse import bacc
import concourse.tile as tile
from concourse.bass_utils import run_bass_kernel_spmd

B, S, EMB = 4, 4096, 2048
HEADS, HD, FFN = 16, 128, 512
T = B * S                      # 16384 tokens
N_CORES = 8
HPC = HEADS // N_CORES         # heads per core = 2
TOK = 256                      # tokens per on-chip tile
NT = T // TOK                  # token tiles per head
NCH = FFN // HD                # ffn chunks of 128 = 4
SLAB = 4096                    # output slab tokens (drained by 2 half DMAs)

F32 = mybir.dt.float32
BF16 = mybir.dt.bfloat16
AF = mybir.ActivationFunctionType


def _build_nc():
    nc = bacc.Bacc("TRN2", target_bir_lowering=False)

    xT = nc.dram_tensor("xT", [HPC, HD, T], BF16, kind="ExternalInput")
    gw = nc.dram_tensor("gw", [HPC, HD, FFN], BF16, kind="ExternalInput")
    uw = nc.dram_tensor("uw", [HPC, HD, FFN], BF16, kind="ExternalInput")
    dw = nc.dram_tensor("dw", [HPC, FFN, HD], BF16, kind="ExternalInput")
    outT = nc.dram_tensor("outT", [HPC, HD, T], F32, kind="ExternalOutput")

    # Output accumulates in SBUF slabs of SLAB tokens, drained by two large
    # read-only DMAs each (one per copy-engine half) so every DMA needs at
    # most ONE semaphore wait (walrus DIRECT2D DMAs only support one).
    TPS = SLAB // TOK  # tiles per slab

    with tile.TileContext(nc) as tc:
        with (
            tc.tile_pool(name="wpool", bufs=1) as wpool,
            tc.tile_pool(name="gpool", bufs=2, space="PSUM") as gpool,
            tc.tile_pool(name="upool", bufs=2, space="PSUM") as upool,
            tc.tile_pool(name="sgpool", bufs=4) as sgpool,
            tc.tile_pool(name="hpool", bufs=4) as hpool,
            tc.tile_pool(name="slabs", bufs=3) as slabs,
        ):
            # weights + the entire x shard resident in SBUF for the kernel
            gw_s = wpool.tile([HD, HPC, FFN], BF16)
            uw_s = wpool.tile([HD, HPC, FFN], BF16)
            dw_s = wpool.tile([HD, HPC, NCH, HD], BF16)
            xs_full = wpool.tile([HD, HPC, T], BF16)
            for h in range(HPC):
                nc.sync.dma_start(out=gw_s[:, h, :], in_=gw[h])
                nc.sync.dma_start(out=uw_s[:, h, :], in_=uw[h])
                nc.sync.dma_start(
                    out=dw_s[:, h, :, :],
                    in_=dw[h].rearrange("(c p) d -> p c d", p=HD),
                )
                # write-once chunked loads; small chunks so tile 0's matmuls
                # start after ~128KB instead of after the whole shard
                XC = 512
                for xc in range(T // XC):
                    c0 = xc * XC
                    nc.sync.dma_start(
                        out=xs_full[:, h, c0 : c0 + XC],
                        in_=xT[h, :, c0 : c0 + XC],
                    )

            # Software pipeline with a 2-tile lag on the down-proj: tile k's
            # down-proj + slab copy are emitted in iteration k+2, when every
            # dependency (hh(k), silu(k+1)'s read of the overlay bank) has
            # already retired, so neither PE nor ACT/DVE ever queue a stalled
            # instruction ahead of ready work.  The down-proj PSUM output is
            # overlaid into the gate-psum banks of tile k+1 (consumed by
            # silu(k+1), recycled by gate(k+3)), keeping total PSUM usage at
            # 8 banks with everything double-buffered.
            slab = None
            pend = []  # [(hh, slab, h, t, o_target), ...] oldest first

            def emit_down_mms(p):
                phh, pslab, ph, pt, ops = p
                for c in range(NCH):
                    nc.tensor.matmul(
                        ops,
                        lhsT=dw_s[:, ph, c, :],
                        rhs=phh[:, c * TOK : (c + 1) * TOK],
                        start=(c == 0),
                        stop=(c == NCH - 1),
                    )

            def emit_copy(p):
                # psum -> slab, always on ScalarE: DVE stays a pure mul
                # pipeline (mul is on the critical psum-recycle chain) and
                # each quarter-slab drain DMA waits on ScalarE alone.
                phh, pslab, ph, pt, ops = p
                pts = pt % TPS
                nc.scalar.copy(pslab[:, pts * TOK : (pts + 1) * TOK], ops)
                QT = TPS // 4  # tiles per drained quarter
                if pts % QT == QT - 1:
                    q = pts // QT
                    pt0 = pt * TOK
                    nc.sync.dma_start(
                        out=outT[ph, :, pt0 + TOK - QT * TOK : pt0 + TOK],
                        in_=pslab[:, q * QT * TOK : (q + 1) * QT * TOK],
                    )

            tiles = [(h, t) for h in range(HPC) for t in range(NT)]
            K = len(tiles)

            def emit_gate(k):
                h, t = tiles[k]
                xs = xs_full[:, h, t * TOK : (t + 1) * TOK]
                gps = gpool.tile([HD, NCH * TOK], F32, name=f"gps_{k}", tag="g")
                for c in range(NCH):
                    nc.tensor.matmul(
                        gps[:, c * TOK : (c + 1) * TOK],
                        lhsT=gw_s[:, h, c * HD : (c + 1) * HD],
                        rhs=xs,
                        start=True,
                        stop=True,
                    )
                sg = sgpool.tile([HD, NCH * TOK], BF16, name=f"sg_{k}", tag="sg")
                nc.scalar.activation(sg[:], gps[:], AF.Silu)
                return gps, sg

            # prologue: gate+silu for tile 0
            gate_next = emit_gate(0)
            for k in range(K):
                h, t = tiles[k]
                if t % TPS == 0:
                    slab = slabs.tile([HD, SLAB], F32, name=f"slab_{k}", tag="slab")

                # tile k-2's down-proj + slab copy: every dependency retired
                if len(pend) == 2:
                    done = pend.pop(0)
                    emit_down_mms(done)
                    emit_copy(done)

                gps, sg = gate_next

                ups = upool.tile([HD, NCH * TOK], F32, name=f"ups_{k}", tag="u")
                xs = xs_full[:, h, t * TOK : (t + 1) * TOK]
                for c in range(NCH):
                    nc.tensor.matmul(
                        ups[:, c * TOK : (c + 1) * TOK],
                        lhsT=uw_s[:, h, c * HD : (c + 1) * HD],
                        rhs=xs,
                        start=True,
                        stop=True,
                    )
                # next tile's gate+silu ahead of this tile's mul: PE runs it
                # during the mul; silu(k+1) overlaps mul(k) on ACT
                if k + 1 < K:
                    gate_next = emit_gate(k + 1)
                hh = hpool.tile([HD, NCH * TOK], BF16, name=f"hh_{k}", tag="hh")
                nc.vector.tensor_mul(hh[:], sg[:], ups[:])

                # tile k-1's down-proj targets THIS tile's gate banks
                if pend:
                    pend[-1] = pend[-1][:4] + (gps[:, :TOK],)
                pend.append((hh, slab, h, t, ups[:, :TOK]))
            # epilogue: last two tiles overlay into their own up banks
            for p in pend:
                emit_down_mms(p)
                emit_copy(p)
    nc.compile()
    return nc


def _shard_inputs(inputs):
    import ml_dtypes

    bf16 = ml_dtypes.bfloat16
    x = np.asarray(inputs["x"], dtype=np.float32)
    gw = np.asarray(inputs["gate_w"], dtype=np.float32).astype(bf16)
    uw = np.asarray(inputs["up_w"], dtype=np.float32).astype(bf16)
    dw = np.asarray(inputs["down_w"], dtype=np.float32).astype(bf16)

    xh = x.reshape(T, HEADS, HD)
    xt = np.ascontiguousarray(xh.transpose(1, 2, 0)).astype(bf16)  # [16, 128, T]

    in_maps = []
    for c in range(N_CORES):
        hs = slice(HPC * c, HPC * (c + 1))
        in_maps.append(
            {
                "xT": xt[hs],
                "gw": gw[hs],
                "uw": uw[hs],
                "dw": dw[hs],
            }
        )
    return in_maps


def run(inputs, trace=False, **spmd_kwargs):
    nc = _build_nc()
    in_maps = _shard_inputs(inputs)
    res = run_bass_kernel_spmd(
        nc, in_maps, core_ids=list(range(N_CORES)), trace=trace, **spmd_kwargs
    )
    outT = np.empty((HEADS, HD, T), dtype=np.float32)
    for c in range(N_CORES):
        outT[HPC * c : HPC * (c + 1)] = res.results[c]["outT"]
    out = np.ascontiguousarray(outT.transpose(2, 0, 1)).reshape(B, S, EMB)
    return out, res


def kernel(**inputs):
    out, _ = run(inputs)
    return out
